# revision 24
# baseline (speedup 1.0000x reference)
"""Trainium2 Bass kernel for nn_EquivariantMLP_68745246540041.

Structure of the reference network: the output Linear only has a path from
the l=0 (scalar) block, and the scalar block of each Gate layer depends only
on the scalar block of its input.  So the live computation is

    y1 = x[:, :64] @ (W0_0[:, :64] * norm)          # (N, 64)
    s1 = CST * silu(y1)
    y2 = s1 @ (W1_0[:, :64] * norm)                 # (N, 64)
    s2 = CST * silu(y2)
    out = s2 @ (W_out * norm)                       # (N, 128)
    result = segment_sum(out, batch_indices, 512)   # (512, 128)

Device strategy (8 NeuronCores):
  - Segments (batch ids) are sharded across cores: core k owns segments
    [64k, 64k+64).  Atoms are grouped by segment on the host and placed into
    L-slot padded bins (zero padding - zeros are fixed points of the whole
    pipeline, so padded slots contribute nothing to the sums).
  - On-chip layout is "transposed + h-folded": partition p = h*64 + m where
    m is the feature index and h in {0,1} picks half of the core's segments.
    Weights become 128x128 block-diagonal matrices so one matmul processes
    both halves with full PE-array contraction width.
  - Per chunk of G=2 segments: matmul (into PSUM bank-aligned slices) ->
    Silu (ScalarE LUT, one wide 3D-AP op) -> matmul -> Silu -> VectorE
    tensor_reduce over each L-slot bin = the per-segment sums.  Double-
    buffered PSUM blocks per stage keep the ScalarE silu stream gap-free.
    The final W_out matmul is applied after the segment reduction
    (64 -> 128 on the 64 reduced columns only, ~nothing).
  - The CST / 1/sqrt(64) constants are folded into the weights on the host.
"""

import numpy as np

import concourse.bass as bass
import concourse.tile as tile
from concourse import mybir
from concourse.bass_utils import run_bass_kernel_spmd

F32 = mybir.dt.float32
F32R = mybir.dt.float32r

N_CORES = 8
H = 64

def _split_waits(nc, maxw: int = 1):
    """walrus' codegen rejects instructions carrying more than `maxw`
    semaphore waits.  Hoist excess waits onto nop instructions inserted
    immediately before the offender on the same engine stream — the engine
    stalls on the nops first, so semantics are identical."""
    for fn in nc.m.functions:
        for bb in fn.blocks:
            insts = bb.instructions
            if not any(
                inst.sync_info is not None
                and inst.sync_info.on_wait
                and len(inst.sync_info.on_wait) > maxw
                for inst in insts
            ):
                continue
            new = []
            for inst in insts:
                si = inst.sync_info
                if si is not None and si.on_wait and len(si.on_wait) > maxw:
                    waits = list(si.on_wait)
                    extra, keep = waits[:-maxw], waits[-maxw:]
                    for i in range(0, len(extra), maxw):
                        nop = mybir.InstNoOp(
                            name=nc.get_next_instruction_name(),
                            engine=inst.engine,
                            sync_info=mybir.SyncInfo(
                                on_wait=extra[i : i + maxw], on_update=[]
                            ),
                            bass_nofuse=True,
                        )
                        new.append(nop)
                    inst.sync_info = mybir.SyncInfo(
                        on_wait=keep,
                        on_update=list(si.on_update) if si.on_update else [],
                    )
                new.append(inst)
            bb.instructions = new


def _cst() -> np.float32:
    # e3nn normalize2mom constant for SiLU, reproduced exactly as in the
    # reference (np.random.default_rng(0), 1e6 samples).
    z = np.random.default_rng(0).standard_normal(1_000_000)
    s = z / (1.0 + np.exp(-z))
    return np.float32(1.0 / np.sqrt(np.mean(s * s)))


def _block_diag2(a: np.ndarray) -> np.ndarray:
    k, m = a.shape
    out = np.zeros((2 * k, 2 * m), np.float32)
    out[:k, :m] = a
    out[k:, m:] = a
    return np.ascontiguousarray(out)


BANK = 512  # PSUM bank width in f32 elements


def _build_program(L: int, s2: int, dtype: str, G: int = 3):
    """Build the SPMD Bass program.

    L: padded bin width per segment (<= 512, one PSUM bank per matmul)
    s2: per-half free width = (segs_per_core/2) * L
    dtype: 'f32' (exact), 'f32r' (PE fast-fp32, ~1e-4), 'bf16' (~1e-3)
    G: segments processed per chunk (psum block = G banks, 2 blocks live)
    """
    n_segs = s2 // L
    # Chunks of G segments; a chunk's G*L slots are processed by bank-packed
    # matmuls (N=512 regardless of segment boundaries - only the reduce is
    # segment-aligned), so a chunk needs ceil(G*L/512) <= 2 PSUM banks and
    # four chunk blocks (two stages, double-buffered) fit the 8 banks.
    assert G * L <= 2 * BANK
    chunks = []
    rem = n_segs
    while rem:
        g = G if rem >= G else rem
        if rem - g == 1:  # avoid a trailing 1-segment chunk
            g -= 1
        chunks.append(g)
        rem -= g
    n_chunks = len(chunks)

    # Tensors feeding f32r matmuls must themselves be declared float32r
    # (the BIR verifier requires producers to round to f32r).  float32r is
    # bit-identical 4-byte storage, so the host still supplies float32.
    FIN = {"f32": F32, "f32r": F32R, "bf16": mybir.dt.bfloat16}[dtype]
    # s1 (silu1 output, mm2 rhs): full-width f32r costs the same on ACT but
    # avoids the bf16 re-quantization of the hidden layer.
    FS1 = F32R if dtype == "bf16" else FIN
    nc = bass.Bass("TRN2", target_bir_lowering=False, debug=False)
    xt_d = nc.dram_tensor("xt", [128, s2], FIN, kind="ExternalInput").ap()
    wa_d = nc.dram_tensor("wa", [128, 128], FIN, kind="ExternalInput").ap()
    wb_d = nc.dram_tensor("wb", [128, 128], FS1, kind="ExternalInput").ap()
    wc0_d = nc.dram_tensor("wc0", [128, 128], F32, kind="ExternalInput").ap()
    wc1_d = nc.dram_tensor("wc1", [128, 128], F32, kind="ExternalInput").ap()
    outa_d = nc.dram_tensor("outa", [128, n_segs], F32, kind="ExternalOutput").ap()
    outb_d = nc.dram_tensor("outb", [128, n_segs], F32, kind="ExternalOutput").ap()

    silu = mybir.ActivationFunctionType.Silu

    with tile.TileContext(nc) as tc:
        with (
            tc.tile_pool(name="w", bufs=1) as wpool,
            tc.tile_pool(name="xin", bufs=3) as xpool,
            tc.tile_pool(name="act", bufs=4) as spool,
            tc.tile_pool(name="ps", bufs=2, space="PSUM") as ppool,
            tc.tile_pool(name="res", bufs=1) as rpool,
        ):
            # Weights ride SWDGE so the HWDGE queue starts streaming x
            # immediately.
            wa = wpool.tile([128, 128], FIN, tag="wa")
            nc.gpsimd.dma_start(wa[:], wa_d[:])
            wb = wpool.tile([128, 128], FS1, tag="wb")
            nc.gpsimd.dma_start(wb[:], wb_d[:])
            wc0 = wpool.tile([128, 128], F32, tag="wc0")
            nc.gpsimd.dma_start(wc0[:], wc0_d[:])
            wc1 = wpool.tile([128, 128], F32, tag="wc1")
            nc.gpsimd.dma_start(wc1[:], wc1_d[:])

            segcols = rpool.tile([128, n_segs], F32, tag="segcols")

            # Chunk slot offsets.
            coff = [0]
            for g in chunks:
                coff.append(coff[-1] + g * L)

            # x is loaded in groups of chunks.  The DGE trigger cost is per
            # partition-row iteration (~0.6us regardless of width), so fewer
            # wider DMAs waste less queue time; the first groups are small so
            # the compute pipeline fills early.
            groups = []
            rem = n_chunks
            for gw in [1, 1, 2] + [4] * n_chunks:
                if rem == 0:
                    break
                gw = min(gw, rem)
                groups.append(gw)
                rem -= gw
            xtiles = {}
            c0 = 0
            for gw in groups:
                lo, hi = coff[c0], coff[c0 + gw]
                xbig = xpool.tile([128, hi - lo], FIN, tag="xin")
                hw = (hi - lo) // 2
                nc.sync.dma_start(xbig[:, :hw], xt_d[:, lo : lo + hw])
                nc.gpsimd.dma_start(xbig[:, hw:], xt_d[:, lo + hw : hi])
                for c in range(c0, c0 + gw):
                    xtiles[c] = (xbig, coff[c] - lo)
                c0 += gw

            segbase = 0
            for j in range(n_chunks):
                g = chunks[j]
                W = g * L
                xbig, xoff = xtiles[j]

                # Bank-packed matmuls: N=512 slices over contiguous slots,
                # each output slice within one PSUM bank.
                yblk1 = ppool.tile([128, 2 * BANK], F32, tag="y1")
                for o in range(0, W, BANK):
                    n = min(BANK, W - o)
                    nc.tensor.matmul(
                        yblk1[:, o : o + n],
                        wa[:],
                        xbig[:, xoff + o : xoff + o + n],
                        start=True,
                        stop=True,
                    )
                s1 = spool.tile([128, W], FS1, tag="s1")
                nc.scalar.activation(s1[:], yblk1[:, 0:W], silu)

                yblk2 = ppool.tile([128, 2 * BANK], F32, tag="y2")
                for o in range(0, W, BANK):
                    n = min(BANK, W - o)
                    nc.tensor.matmul(
                        yblk2[:, o : o + n],
                        wb[:],
                        s1[:, o : o + n],
                        start=True,
                        stop=True,
                    )
                s2t = spool.tile([128, W], F32, tag="s2")
                nc.scalar.activation(s2t[:], yblk2[:, 0:W], silu)

                nc.vector.tensor_reduce(
                    segcols[:, segbase : segbase + g],
                    s2t[:].rearrange("p (g l) -> p g l", l=L),
                    axis=mybir.AxisListType.X,
                    op=mybir.AluOpType.add,
                )
                segbase += g

            # Final W_out matmuls, split into column halves so the first
            # half overlaps the tail of the chunk loop.
            oblk = ppool.tile([128, 2 * BANK], F32, tag="y1")
            oa = rpool.tile([128, n_segs], F32, tag="oa")
            ob = rpool.tile([128, n_segs], F32, tag="ob")
            hseg = n_segs // 2
            for c0_, c1_ in ((0, hseg), (hseg, n_segs)):
                nc.tensor.matmul(
                    oblk[:, c0_:c1_],
                    wc0[:],
                    segcols[:, c0_:c1_],
                    start=True,
                    stop=True,
                )
                nc.tensor.matmul(
                    oblk[:, BANK + c0_ : BANK + c1_],
                    wc1[:],
                    segcols[:, c0_:c1_],
                    start=True,
                    stop=True,
                )
                nc.vector.tensor_copy(oa[:, c0_:c1_], oblk[:, c0_:c1_])
                nc.vector.tensor_copy(
                    ob[:, c0_:c1_], oblk[:, BANK + c0_ : BANK + c1_]
                )
            nc.sync.dma_start(outa_d[:], oa[:])
            nc.sync.dma_start(outb_d[:], ob[:])

    _split_waits(nc)
    return nc


def _prepare(x, batch_indices, W0_0, W1_0, W_out, batch_size, dtype="f32"):
    """Host-side layout: shard segments across cores, bin atoms into padded
    per-segment slots, transpose + h-fold, fold constants into weights."""
    B = int(batch_size)
    N = x.shape[0]
    assert B % N_CORES == 0
    segs_per_core = B // N_CORES
    assert segs_per_core % 2 == 0
    half = segs_per_core // 2

    bi = np.asarray(batch_indices).astype(np.int64).ravel()
    assert bi.shape[0] == N

    sizes = np.bincount(bi, minlength=B)
    maxseg = int(sizes.max())
    L = max(256, -(-maxseg // 64) * 64)
    assert L <= 512, f"segment of size {maxseg} exceeds supported bin width"
    s2 = half * L

    order = np.argsort(bi, kind="stable")
    starts = np.zeros(B + 1, np.int64)
    starts[1:] = np.cumsum(sizes)
    bi_sorted = bi[order]
    ranks = np.arange(N, dtype=np.int64) - starts[bi_sorted]
    dest = bi_sorted * L + ranks

    x64 = np.ascontiguousarray(np.asarray(x, dtype=np.float32)[:, :H])
    Xp = np.zeros((B * L, H), np.float32)
    Xp[dest] = x64[order]
    # (core, h, s2, m) -> (core, h, m, s2) -> (core, 128, s2)
    xt_all = np.ascontiguousarray(
        Xp.reshape(N_CORES, 2, s2, H).transpose(0, 1, 3, 2)
    ).reshape(N_CORES, 128, s2)

    norm = np.float32(1.0 / np.sqrt(H))
    cst = _cst()
    A = (np.asarray(W0_0, np.float32)[:, :H] * norm).astype(np.float32)
    Bw = (np.asarray(W1_0, np.float32)[:, :H] * (norm * cst)).astype(np.float32)
    C = (np.asarray(W_out, np.float32) * (norm * cst)).astype(np.float32)
    bdA = _block_diag2(A)
    bdB = _block_diag2(Bw)
    bdC0 = _block_diag2(C[:, :H])
    bdC1 = _block_diag2(C[:, H:])

    if dtype == "bf16":
        import ml_dtypes

        bf16 = np.dtype(ml_dtypes.bfloat16)
        xt_all = np.ascontiguousarray(xt_all.astype(bf16))
        bdA = bdA.astype(bf16)

    in_maps = [
        {
            "xt": xt_all[k],
            "wa": bdA,
            "wb": bdB,
            "wc0": bdC0,
            "wc1": bdC1,
        }
        for k in range(N_CORES)
    ]
    return in_maps, L, s2, half, B


def _assemble(results, half, B):
    out = np.zeros((B, 2 * H), np.float32)
    for k in range(N_CORES):
        oa = results[k]["outa"]
        ob = results[k]["outb"]
        for h in range(2):
            rows = slice(2 * half * k + h * half, 2 * half * k + (h + 1) * half)
            out[rows, :H] = oa[h * H : (h + 1) * H, :].T
            out[rows, H:] = ob[h * H : (h + 1) * H, :].T
    return out


class _LdwOpt:
    """Enable walrus' redundant-LDWEIGHTS elision for this kernel's compile.
    Both matmul stages reuse one stationary operand across sub-segments, so
    half the weight loads are no-ops; the conservative default leaves them
    in.  Correctness is verified against the reference output downstream."""

    def __enter__(self):
        import concourse.bass_utils as bu

        self._orig = bu.run_command

        def patched(argv, **kw):
            argv = [
                a.replace("--enable-ldw-opt=false", "--enable-ldw-opt=true")
                if isinstance(a, str)
                else a
                for a in argv
            ]
            return self._orig(argv, **kw)

        bu.run_command = patched
        return self

    def __exit__(self, *exc):
        import concourse.bass_utils as bu

        bu.run_command = self._orig


def run(
    inputs: dict,
    dtype: str = "f32",
    trace: bool = False,
    ldw_opt: bool = False,
    **run_kwargs,
):
    in_maps, L, s2, half, B = _prepare(
        inputs["x"],
        inputs["batch_indices"],
        inputs["W0_0"],
        inputs["W1_0"],
        inputs["W_out"],
        inputs["batch_size"],
        dtype=dtype,
    )
    nc = _build_program(L, s2, dtype)
    import contextlib

    with _LdwOpt() if ldw_opt else contextlib.nullcontext():
        res = run_bass_kernel_spmd(
            nc, in_maps, core_ids=list(range(N_CORES)), trace=trace, **run_kwargs
        )
    out = _assemble(res.results, half, B)
    return out, res


def kernel(**inputs) -> np.ndarray:
    out, _ = run(inputs)
    return out


# revision 25
# speedup vs baseline: 1.2331x; 1.2331x over previous
"""Trainium2 Bass kernel for nn_EquivariantMLP_68745246540041.

Structure of the reference network: the output Linear only has a path from
the l=0 (scalar) block, and the scalar block of each Gate layer depends only
on the scalar block of its input.  So the live computation is

    y1 = x[:, :64] @ (W0_0[:, :64] * norm)          # (N, 64)
    s1 = CST * silu(y1)
    y2 = s1 @ (W1_0[:, :64] * norm)                 # (N, 64)
    s2 = CST * silu(y2)
    out = s2 @ (W_out * norm)                       # (N, 128)
    result = segment_sum(out, batch_indices, 512)   # (512, 128)

Device strategy (8 NeuronCores):
  - Segments (batch ids) are sharded across cores: core k owns segments
    [64k, 64k+64).  Atoms are grouped by segment on the host and placed into
    L-slot padded bins (zero padding - zeros are fixed points of the whole
    pipeline, so padded slots contribute nothing to the sums).
  - On-chip layout is "transposed + h-folded": partition p = h*64 + m where
    m is the feature index and h in {0,1} picks half of the core's segments.
    Weights become 128x128 block-diagonal matrices so one matmul processes
    both halves with full PE-array contraction width.
  - Per chunk of G=2 segments: matmul (into PSUM bank-aligned slices) ->
    Silu (ScalarE LUT, one wide 3D-AP op) -> matmul -> Silu -> VectorE
    tensor_reduce over each L-slot bin = the per-segment sums.  Double-
    buffered PSUM blocks per stage keep the ScalarE silu stream gap-free.
    The final W_out matmul is applied after the segment reduction
    (64 -> 128 on the 64 reduced columns only, ~nothing).
  - The CST / 1/sqrt(64) constants are folded into the weights on the host.
"""

import numpy as np

import concourse.bass as bass
import concourse.tile as tile
from concourse import mybir
from concourse.bass_utils import run_bass_kernel_spmd

F32 = mybir.dt.float32
F32R = mybir.dt.float32r

N_CORES = 8
H = 64

def _split_waits(nc, maxw: int = 1):
    """walrus' codegen rejects instructions carrying more than `maxw`
    semaphore waits.  Hoist excess waits onto nop instructions inserted
    immediately before the offender on the same engine stream — the engine
    stalls on the nops first, so semantics are identical."""
    for fn in nc.m.functions:
        for bb in fn.blocks:
            insts = bb.instructions
            if not any(
                inst.sync_info is not None
                and inst.sync_info.on_wait
                and len(inst.sync_info.on_wait) > maxw
                for inst in insts
            ):
                continue
            new = []
            for inst in insts:
                si = inst.sync_info
                if si is not None and si.on_wait and len(si.on_wait) > maxw:
                    waits = list(si.on_wait)
                    extra, keep = waits[:-maxw], waits[-maxw:]
                    for i in range(0, len(extra), maxw):
                        nop = mybir.InstNoOp(
                            name=nc.get_next_instruction_name(),
                            engine=inst.engine,
                            sync_info=mybir.SyncInfo(
                                on_wait=extra[i : i + maxw], on_update=[]
                            ),
                            bass_nofuse=True,
                        )
                        new.append(nop)
                    inst.sync_info = mybir.SyncInfo(
                        on_wait=keep,
                        on_update=list(si.on_update) if si.on_update else [],
                    )
                new.append(inst)
            bb.instructions = new


def _cst() -> np.float32:
    # e3nn normalize2mom constant for SiLU, reproduced exactly as in the
    # reference (np.random.default_rng(0), 1e6 samples).
    z = np.random.default_rng(0).standard_normal(1_000_000)
    s = z / (1.0 + np.exp(-z))
    return np.float32(1.0 / np.sqrt(np.mean(s * s)))


def _block_diag2(a: np.ndarray) -> np.ndarray:
    k, m = a.shape
    out = np.zeros((2 * k, 2 * m), np.float32)
    out[:k, :m] = a
    out[k:, m:] = a
    return np.ascontiguousarray(out)


BANK = 512  # PSUM bank width in f32 elements


def _build_program(L: int, s2: int, dtype: str, G: int = 2):
    """Build the SPMD Bass program.

    L: padded bin width per segment (<= 512, one PSUM bank per matmul)
    s2: per-half free width = (segs_per_core/2) * L
    dtype: 'f32' (exact), 'f32r' (PE fast-fp32, ~1e-4), 'bf16' (~1e-3)
    G: segments processed per chunk (psum block = G banks, 2 blocks live)
    """
    n_segs = s2 // L
    # Chunks of G segments; a chunk's G*L slots are processed by bank-packed
    # matmuls (N=512 regardless of segment boundaries - only the reduce is
    # segment-aligned), so a chunk needs ceil(G*L/512) <= 2 PSUM banks and
    # four chunk blocks (two stages, double-buffered) fit the 8 banks.
    assert G * L <= 2 * BANK
    chunks = []
    rem = n_segs
    while rem:
        g = G if rem >= G else rem
        if rem - g == 1:  # avoid a trailing 1-segment chunk
            g -= 1
        chunks.append(g)
        rem -= g
    n_chunks = len(chunks)

    # Tensors feeding f32r matmuls must themselves be declared float32r
    # (the BIR verifier requires producers to round to f32r).  float32r is
    # bit-identical 4-byte storage, so the host still supplies float32.
    FIN = {"f32": F32, "f32r": F32R, "bf16": mybir.dt.bfloat16}[dtype]
    # s1 (silu1 output, mm2 rhs): full-width f32r costs the same on ACT but
    # avoids the bf16 re-quantization of the hidden layer.
    FS1 = F32R if dtype == "bf16" else FIN
    nc = bass.Bass("TRN2", target_bir_lowering=False, debug=False)
    xt_d = nc.dram_tensor("xt", [128, s2], FIN, kind="ExternalInput").ap()
    wa_d = nc.dram_tensor("wa", [128, 128], FIN, kind="ExternalInput").ap()
    wb_d = nc.dram_tensor("wb", [128, 128], FS1, kind="ExternalInput").ap()
    wc0_d = nc.dram_tensor("wc0", [128, 128], F32, kind="ExternalInput").ap()
    wc1_d = nc.dram_tensor("wc1", [128, 128], F32, kind="ExternalInput").ap()
    outa_d = nc.dram_tensor("outa", [128, n_segs], F32, kind="ExternalOutput").ap()
    outb_d = nc.dram_tensor("outb", [128, n_segs], F32, kind="ExternalOutput").ap()

    silu = mybir.ActivationFunctionType.Silu

    with tile.TileContext(nc) as tc:
        with (
            tc.tile_pool(name="w", bufs=1) as wpool,
            tc.tile_pool(name="xin", bufs=3) as xpool,
            tc.tile_pool(name="act", bufs=4) as spool,
            tc.tile_pool(name="ps", bufs=2, space="PSUM") as ppool,
            tc.tile_pool(name="res", bufs=1) as rpool,
        ):
            # Weights ride SWDGE so the HWDGE queue starts streaming x
            # immediately.
            wa = wpool.tile([128, 128], FIN, tag="wa")
            nc.gpsimd.dma_start(wa[:], wa_d[:])
            wb = wpool.tile([128, 128], FS1, tag="wb")
            nc.gpsimd.dma_start(wb[:], wb_d[:])
            wc0 = wpool.tile([128, 128], F32, tag="wc0")
            nc.gpsimd.dma_start(wc0[:], wc0_d[:])
            wc1 = wpool.tile([128, 128], F32, tag="wc1")
            nc.gpsimd.dma_start(wc1[:], wc1_d[:])

            segcols = rpool.tile([128, n_segs], F32, tag="segcols")

            # Chunk slot offsets.
            coff = [0]
            for g in chunks:
                coff.append(coff[-1] + g * L)

            # x is loaded in groups of chunks.  The DGE trigger cost is per
            # partition-row iteration (~0.6us regardless of width), so fewer
            # wider DMAs waste less queue time; the first groups are small so
            # the compute pipeline fills early.
            groups = []
            rem = n_chunks
            for gw in [1, 1, 2] + [4] * n_chunks:
                if rem == 0:
                    break
                gw = min(gw, rem)
                groups.append(gw)
                rem -= gw
            xtiles = {}
            c0 = 0
            for gw in groups:
                lo, hi = coff[c0], coff[c0 + gw]
                xbig = xpool.tile([128, hi - lo], FIN, tag="xin")
                hw = (hi - lo) // 2
                nc.sync.dma_start(xbig[:, :hw], xt_d[:, lo : lo + hw])
                nc.gpsimd.dma_start(xbig[:, hw:], xt_d[:, lo + hw : hi])
                for c in range(c0, c0 + gw):
                    xtiles[c] = (xbig, coff[c] - lo)
                c0 += gw

            segbase = 0
            for j in range(n_chunks):
                g = chunks[j]
                W = g * L
                xbig, xoff = xtiles[j]

                # Bank-packed matmuls: N=512 slices over contiguous slots,
                # each output slice within one PSUM bank.
                yblk1 = ppool.tile([128, 2 * BANK], F32, tag="y1")
                for o in range(0, W, BANK):
                    n = min(BANK, W - o)
                    nc.tensor.matmul(
                        yblk1[:, o : o + n],
                        wa[:],
                        xbig[:, xoff + o : xoff + o + n],
                        start=True,
                        stop=True,
                    )
                s1 = spool.tile([128, W], FS1, tag="s1")
                nc.scalar.activation(s1[:], yblk1[:, 0:W], silu)

                yblk2 = ppool.tile([128, 2 * BANK], F32, tag="y2")
                for o in range(0, W, BANK):
                    n = min(BANK, W - o)
                    nc.tensor.matmul(
                        yblk2[:, o : o + n],
                        wb[:],
                        s1[:, o : o + n],
                        start=True,
                        stop=True,
                    )
                s2t = spool.tile([128, W], F32, tag="s2")
                nc.scalar.activation(s2t[:], yblk2[:, 0:W], silu)

                nc.vector.tensor_reduce(
                    segcols[:, segbase : segbase + g],
                    s2t[:].rearrange("p (g l) -> p g l", l=L),
                    axis=mybir.AxisListType.X,
                    op=mybir.AluOpType.add,
                )
                segbase += g

            # Final W_out matmuls, split into column halves so the first
            # half overlaps the tail of the chunk loop.
            oblk = ppool.tile([128, 2 * BANK], F32, tag="y1")
            oa = rpool.tile([128, n_segs], F32, tag="oa")
            ob = rpool.tile([128, n_segs], F32, tag="ob")
            hseg = n_segs // 2
            for c0_, c1_ in ((0, hseg), (hseg, n_segs)):
                nc.tensor.matmul(
                    oblk[:, c0_:c1_],
                    wc0[:],
                    segcols[:, c0_:c1_],
                    start=True,
                    stop=True,
                )
                nc.tensor.matmul(
                    oblk[:, BANK + c0_ : BANK + c1_],
                    wc1[:],
                    segcols[:, c0_:c1_],
                    start=True,
                    stop=True,
                )
                nc.vector.tensor_copy(oa[:, c0_:c1_], oblk[:, c0_:c1_])
                nc.vector.tensor_copy(
                    ob[:, c0_:c1_], oblk[:, BANK + c0_ : BANK + c1_]
                )
            nc.sync.dma_start(outa_d[:], oa[:])
            nc.sync.dma_start(outb_d[:], ob[:])

    _split_waits(nc)
    return nc


def _prepare(x, batch_indices, W0_0, W1_0, W_out, batch_size, dtype="f32"):
    """Host-side layout: shard segments across cores, bin atoms into padded
    per-segment slots, transpose + h-fold, fold constants into weights."""
    B = int(batch_size)
    N = x.shape[0]
    assert B % N_CORES == 0
    segs_per_core = B // N_CORES
    assert segs_per_core % 2 == 0
    half = segs_per_core // 2

    bi = np.asarray(batch_indices).astype(np.int64).ravel()
    assert bi.shape[0] == N

    sizes = np.bincount(bi, minlength=B)
    maxseg = int(sizes.max())
    L = max(256, -(-maxseg // 64) * 64)
    assert L <= 512, f"segment of size {maxseg} exceeds supported bin width"
    s2 = half * L

    order = np.argsort(bi, kind="stable")
    starts = np.zeros(B + 1, np.int64)
    starts[1:] = np.cumsum(sizes)
    bi_sorted = bi[order]
    ranks = np.arange(N, dtype=np.int64) - starts[bi_sorted]
    dest = bi_sorted * L + ranks

    x64 = np.ascontiguousarray(np.asarray(x, dtype=np.float32)[:, :H])
    Xp = np.zeros((B * L, H), np.float32)
    Xp[dest] = x64[order]
    # (core, h, s2, m) -> (core, h, m, s2) -> (core, 128, s2)
    xt_all = np.ascontiguousarray(
        Xp.reshape(N_CORES, 2, s2, H).transpose(0, 1, 3, 2)
    ).reshape(N_CORES, 128, s2)

    norm = np.float32(1.0 / np.sqrt(H))
    cst = _cst()
    A = (np.asarray(W0_0, np.float32)[:, :H] * norm).astype(np.float32)
    Bw = (np.asarray(W1_0, np.float32)[:, :H] * (norm * cst)).astype(np.float32)
    C = (np.asarray(W_out, np.float32) * (norm * cst)).astype(np.float32)
    bdA = _block_diag2(A)
    bdB = _block_diag2(Bw)
    bdC0 = _block_diag2(C[:, :H])
    bdC1 = _block_diag2(C[:, H:])

    if dtype == "bf16":
        import ml_dtypes

        bf16 = np.dtype(ml_dtypes.bfloat16)
        xt_all = np.ascontiguousarray(xt_all.astype(bf16))
        bdA = bdA.astype(bf16)

    in_maps = [
        {
            "xt": xt_all[k],
            "wa": bdA,
            "wb": bdB,
            "wc0": bdC0,
            "wc1": bdC1,
        }
        for k in range(N_CORES)
    ]
    return in_maps, L, s2, half, B


def _assemble(results, half, B):
    out = np.zeros((B, 2 * H), np.float32)
    for k in range(N_CORES):
        oa = results[k]["outa"]
        ob = results[k]["outb"]
        for h in range(2):
            rows = slice(2 * half * k + h * half, 2 * half * k + (h + 1) * half)
            out[rows, :H] = oa[h * H : (h + 1) * H, :].T
            out[rows, H:] = ob[h * H : (h + 1) * H, :].T
    return out


class _LdwOpt:
    """Enable walrus' redundant-LDWEIGHTS elision for this kernel's compile.
    Both matmul stages reuse one stationary operand across sub-segments, so
    half the weight loads are no-ops; the conservative default leaves them
    in.  Correctness is verified against the reference output downstream."""

    def __enter__(self):
        import concourse.bass_utils as bu

        self._orig = bu.run_command

        def patched(argv, **kw):
            argv = [
                a.replace("--enable-ldw-opt=false", "--enable-ldw-opt=true")
                if isinstance(a, str)
                else a
                for a in argv
            ]
            return self._orig(argv, **kw)

        bu.run_command = patched
        return self

    def __exit__(self, *exc):
        import concourse.bass_utils as bu

        bu.run_command = self._orig


def run(
    inputs: dict,
    dtype: str = "f32",
    trace: bool = False,
    ldw_opt: bool = False,
    **run_kwargs,
):
    in_maps, L, s2, half, B = _prepare(
        inputs["x"],
        inputs["batch_indices"],
        inputs["W0_0"],
        inputs["W1_0"],
        inputs["W_out"],
        inputs["batch_size"],
        dtype=dtype,
    )
    nc = _build_program(L, s2, dtype)
    import contextlib

    with _LdwOpt() if ldw_opt else contextlib.nullcontext():
        res = run_bass_kernel_spmd(
            nc, in_maps, core_ids=list(range(N_CORES)), trace=trace, **run_kwargs
        )
    out = _assemble(res.results, half, B)
    return out, res


def kernel(**inputs) -> np.ndarray:
    out, _ = run(inputs)
    return out


# revision 26
# speedup vs baseline: 1.2803x; 1.0383x over previous
"""Trainium2 Bass kernel for nn_EquivariantMLP_68745246540041.

Structure of the reference network: the output Linear only has a path from
the l=0 (scalar) block, and the scalar block of each Gate layer depends only
on the scalar block of its input.  So the live computation is

    y1 = x[:, :64] @ (W0_0[:, :64] * norm)          # (N, 64)
    s1 = CST * silu(y1)
    y2 = s1 @ (W1_0[:, :64] * norm)                 # (N, 64)
    s2 = CST * silu(y2)
    out = s2 @ (W_out * norm)                       # (N, 128)
    result = segment_sum(out, batch_indices, 512)   # (512, 128)

Device strategy (8 NeuronCores):
  - Segments (batch ids) are sharded across cores: core k owns segments
    [64k, 64k+64).  Atoms are grouped by segment on the host and placed into
    L-slot padded bins (zero padding - zeros are fixed points of the whole
    pipeline, so padded slots contribute nothing to the sums).
  - On-chip layout is "transposed + h-folded": partition p = h*64 + m where
    m is the feature index and h in {0,1} picks half of the core's segments.
    Weights become 128x128 block-diagonal matrices so one matmul processes
    both halves with full PE-array contraction width.
  - Per chunk of G=2 segments: matmul (into PSUM bank-aligned slices) ->
    Silu (ScalarE LUT, one wide 3D-AP op) -> matmul -> Silu -> VectorE
    tensor_reduce over each L-slot bin = the per-segment sums.  Double-
    buffered PSUM blocks per stage keep the ScalarE silu stream gap-free.
    The final W_out matmul is applied after the segment reduction
    (64 -> 128 on the 64 reduced columns only, ~nothing).
  - The CST / 1/sqrt(64) constants are folded into the weights on the host.
"""

import numpy as np

import concourse.bass as bass
import concourse.tile as tile
from concourse import mybir
from concourse.bass_utils import run_bass_kernel_spmd

F32 = mybir.dt.float32
F32R = mybir.dt.float32r

N_CORES = 8
H = 64

def _split_waits(nc, maxw: int = 1):
    """walrus' codegen rejects instructions carrying more than `maxw`
    semaphore waits.  Hoist excess waits onto nop instructions inserted
    immediately before the offender on the same engine stream — the engine
    stalls on the nops first, so semantics are identical."""
    for fn in nc.m.functions:
        for bb in fn.blocks:
            insts = bb.instructions
            if not any(
                inst.sync_info is not None
                and inst.sync_info.on_wait
                and len(inst.sync_info.on_wait) > maxw
                for inst in insts
            ):
                continue
            new = []
            for inst in insts:
                si = inst.sync_info
                if si is not None and si.on_wait and len(si.on_wait) > maxw:
                    waits = list(si.on_wait)
                    extra, keep = waits[:-maxw], waits[-maxw:]
                    for i in range(0, len(extra), maxw):
                        nop = mybir.InstNoOp(
                            name=nc.get_next_instruction_name(),
                            engine=inst.engine,
                            sync_info=mybir.SyncInfo(
                                on_wait=extra[i : i + maxw], on_update=[]
                            ),
                            bass_nofuse=True,
                        )
                        new.append(nop)
                    inst.sync_info = mybir.SyncInfo(
                        on_wait=keep,
                        on_update=list(si.on_update) if si.on_update else [],
                    )
                new.append(inst)
            bb.instructions = new


def _cst() -> np.float32:
    # e3nn normalize2mom constant for SiLU, reproduced exactly as in the
    # reference (np.random.default_rng(0), 1e6 samples).
    z = np.random.default_rng(0).standard_normal(1_000_000)
    s = z / (1.0 + np.exp(-z))
    return np.float32(1.0 / np.sqrt(np.mean(s * s)))


def _block_diag2(a: np.ndarray) -> np.ndarray:
    k, m = a.shape
    out = np.zeros((2 * k, 2 * m), np.float32)
    out[:k, :m] = a
    out[k:, m:] = a
    return np.ascontiguousarray(out)


BANK = 512  # PSUM bank width in f32 elements


def _build_program(L: int, s2: int, dtype: str, G: int = 2):
    """Build the SPMD Bass program.

    L: padded bin width per segment (<= 512, one PSUM bank per matmul)
    s2: per-half free width = (segs_per_core/2) * L
    dtype: 'f32r' (default: PE fast-fp32, rel ~1e-4), 'f32' (exact), 'bf16'
    G: segments processed per chunk (psum block = G banks, 2 blocks live)
    """
    n_segs = s2 // L
    # Chunks of G segments; a chunk's G*L slots are processed by bank-packed
    # matmuls (N=512 regardless of segment boundaries - only the reduce is
    # segment-aligned), so a chunk needs ceil(G*L/512) <= 2 PSUM banks and
    # four chunk blocks (two stages, double-buffered) fit the 8 banks.
    assert G * L <= 2 * BANK
    chunks = []
    rem = n_segs
    while rem:
        g = G if rem >= G else rem
        if rem - g == 1:  # avoid a trailing 1-segment chunk
            g -= 1
        chunks.append(g)
        rem -= g
    n_chunks = len(chunks)

    # Tensors feeding f32r matmuls must themselves be declared float32r
    # (the BIR verifier requires producers to round to f32r).  float32r is
    # bit-identical 4-byte storage, so the host still supplies float32.
    FIN = {"f32": F32, "f32r": F32R, "bf16": mybir.dt.bfloat16}[dtype]
    # s1 (silu1 output, mm2 rhs): full-width f32r costs the same on ACT but
    # avoids the bf16 re-quantization of the hidden layer.
    FS1 = F32R if dtype == "bf16" else FIN
    nc = bass.Bass("TRN2", target_bir_lowering=False, debug=False)
    xt_d = nc.dram_tensor("xt", [128, s2], FIN, kind="ExternalInput").ap()
    wa_d = nc.dram_tensor("wa", [128, 128], FIN, kind="ExternalInput").ap()
    wb_d = nc.dram_tensor("wb", [128, 128], FS1, kind="ExternalInput").ap()
    wc0_d = nc.dram_tensor("wc0", [128, 128], F32, kind="ExternalInput").ap()
    wc1_d = nc.dram_tensor("wc1", [128, 128], F32, kind="ExternalInput").ap()
    outa_d = nc.dram_tensor("outa", [128, n_segs], F32, kind="ExternalOutput").ap()
    outb_d = nc.dram_tensor("outb", [128, n_segs], F32, kind="ExternalOutput").ap()

    silu = mybir.ActivationFunctionType.Silu

    with tile.TileContext(nc) as tc:
        with (
            tc.tile_pool(name="w", bufs=1) as wpool,
            tc.tile_pool(name="xin", bufs=3) as xpool,
            tc.tile_pool(name="act", bufs=4) as spool,
            tc.tile_pool(name="ps", bufs=2, space="PSUM") as ppool,
            tc.tile_pool(name="res", bufs=1) as rpool,
        ):
            # Weights ride SWDGE so the HWDGE queue starts streaming x
            # immediately.
            wa = wpool.tile([128, 128], FIN, tag="wa")
            nc.gpsimd.dma_start(wa[:], wa_d[:])
            wb = wpool.tile([128, 128], FS1, tag="wb")
            nc.gpsimd.dma_start(wb[:], wb_d[:])
            wc0 = wpool.tile([128, 128], F32, tag="wc0")
            nc.gpsimd.dma_start(wc0[:], wc0_d[:])
            wc1 = wpool.tile([128, 128], F32, tag="wc1")
            nc.gpsimd.dma_start(wc1[:], wc1_d[:])

            segcols = rpool.tile([128, n_segs], F32, tag="segcols")

            # Chunk slot offsets.
            coff = [0]
            for g in chunks:
                coff.append(coff[-1] + g * L)

            # x is loaded in groups of chunks.  The DGE trigger cost is per
            # partition-row iteration (~0.6us regardless of width), so fewer
            # wider DMAs waste less queue time; the first groups are small so
            # the compute pipeline fills early.
            groups = []
            rem = n_chunks
            for gw in [1, 1, 2] + [4] * n_chunks:
                if rem == 0:
                    break
                gw = min(gw, rem)
                groups.append(gw)
                rem -= gw
            xtiles = {}
            c0 = 0
            for gw in groups:
                lo, hi = coff[c0], coff[c0 + gw]
                xbig = xpool.tile([128, hi - lo], FIN, tag="xin")
                hw = (hi - lo) // 2
                nc.sync.dma_start(xbig[:, :hw], xt_d[:, lo : lo + hw])
                nc.gpsimd.dma_start(xbig[:, hw:], xt_d[:, lo + hw : hi])
                for c in range(c0, c0 + gw):
                    xtiles[c] = (xbig, coff[c] - lo)
                c0 += gw

            segbase = 0
            for j in range(n_chunks):
                g = chunks[j]
                W = g * L
                xbig, xoff = xtiles[j]

                # Bank-packed matmuls: N=512 slices over contiguous slots,
                # each output slice within one PSUM bank.
                yblk1 = ppool.tile([128, 2 * BANK], F32, tag="y1")
                for o in range(0, W, BANK):
                    n = min(BANK, W - o)
                    nc.tensor.matmul(
                        yblk1[:, o : o + n],
                        wa[:],
                        xbig[:, xoff + o : xoff + o + n],
                        start=True,
                        stop=True,
                    )
                s1 = spool.tile([128, W], FS1, tag="s1")
                nc.scalar.activation(s1[:], yblk1[:, 0:W], silu)

                yblk2 = ppool.tile([128, 2 * BANK], F32, tag="y2")
                for o in range(0, W, BANK):
                    n = min(BANK, W - o)
                    nc.tensor.matmul(
                        yblk2[:, o : o + n],
                        wb[:],
                        s1[:, o : o + n],
                        start=True,
                        stop=True,
                    )
                s2t = spool.tile([128, W], F32, tag="s2")
                nc.scalar.activation(s2t[:], yblk2[:, 0:W], silu)

                nc.vector.tensor_reduce(
                    segcols[:, segbase : segbase + g],
                    s2t[:].rearrange("p (g l) -> p g l", l=L),
                    axis=mybir.AxisListType.X,
                    op=mybir.AluOpType.add,
                )
                segbase += g

            # Final W_out matmuls, split into column halves so the first
            # half overlaps the tail of the chunk loop.
            oblk = ppool.tile([128, 2 * BANK], F32, tag="y1")
            oa = rpool.tile([128, n_segs], F32, tag="oa")
            ob = rpool.tile([128, n_segs], F32, tag="ob")
            hseg = n_segs // 2
            for c0_, c1_ in ((0, hseg), (hseg, n_segs)):
                nc.tensor.matmul(
                    oblk[:, c0_:c1_],
                    wc0[:],
                    segcols[:, c0_:c1_],
                    start=True,
                    stop=True,
                )
                nc.tensor.matmul(
                    oblk[:, BANK + c0_ : BANK + c1_],
                    wc1[:],
                    segcols[:, c0_:c1_],
                    start=True,
                    stop=True,
                )
                nc.vector.tensor_copy(oa[:, c0_:c1_], oblk[:, c0_:c1_])
                nc.vector.tensor_copy(
                    ob[:, c0_:c1_], oblk[:, BANK + c0_ : BANK + c1_]
                )
            nc.sync.dma_start(outa_d[:], oa[:])
            nc.sync.dma_start(outb_d[:], ob[:])

    _split_waits(nc)
    return nc


def _prepare(x, batch_indices, W0_0, W1_0, W_out, batch_size, dtype="f32"):
    """Host-side layout: shard segments across cores, bin atoms into padded
    per-segment slots, transpose + h-fold, fold constants into weights."""
    B = int(batch_size)
    N = x.shape[0]
    assert B % N_CORES == 0
    segs_per_core = B // N_CORES
    assert segs_per_core % 2 == 0
    half = segs_per_core // 2

    bi = np.asarray(batch_indices).astype(np.int64).ravel()
    assert bi.shape[0] == N

    sizes = np.bincount(bi, minlength=B)
    maxseg = int(sizes.max())
    L = max(256, -(-maxseg // 64) * 64)
    assert L <= 512, f"segment of size {maxseg} exceeds supported bin width"
    s2 = half * L

    order = np.argsort(bi, kind="stable")
    starts = np.zeros(B + 1, np.int64)
    starts[1:] = np.cumsum(sizes)
    bi_sorted = bi[order]
    ranks = np.arange(N, dtype=np.int64) - starts[bi_sorted]
    dest = bi_sorted * L + ranks

    x64 = np.ascontiguousarray(np.asarray(x, dtype=np.float32)[:, :H])
    Xp = np.zeros((B * L, H), np.float32)
    Xp[dest] = x64[order]
    # (core, h, s2, m) -> (core, h, m, s2) -> (core, 128, s2)
    xt_all = np.ascontiguousarray(
        Xp.reshape(N_CORES, 2, s2, H).transpose(0, 1, 3, 2)
    ).reshape(N_CORES, 128, s2)

    norm = np.float32(1.0 / np.sqrt(H))
    cst = _cst()
    A = (np.asarray(W0_0, np.float32)[:, :H] * norm).astype(np.float32)
    Bw = (np.asarray(W1_0, np.float32)[:, :H] * (norm * cst)).astype(np.float32)
    C = (np.asarray(W_out, np.float32) * (norm * cst)).astype(np.float32)
    bdA = _block_diag2(A)
    bdB = _block_diag2(Bw)
    bdC0 = _block_diag2(C[:, :H])
    bdC1 = _block_diag2(C[:, H:])

    if dtype == "bf16":
        import ml_dtypes

        bf16 = np.dtype(ml_dtypes.bfloat16)
        xt_all = np.ascontiguousarray(xt_all.astype(bf16))
        bdA = bdA.astype(bf16)

    in_maps = [
        {
            "xt": xt_all[k],
            "wa": bdA,
            "wb": bdB,
            "wc0": bdC0,
            "wc1": bdC1,
        }
        for k in range(N_CORES)
    ]
    return in_maps, L, s2, half, B


def _assemble(results, half, B):
    out = np.zeros((B, 2 * H), np.float32)
    for k in range(N_CORES):
        oa = results[k]["outa"]
        ob = results[k]["outb"]
        for h in range(2):
            rows = slice(2 * half * k + h * half, 2 * half * k + (h + 1) * half)
            out[rows, :H] = oa[h * H : (h + 1) * H, :].T
            out[rows, H:] = ob[h * H : (h + 1) * H, :].T
    return out


class _LdwOpt:
    """Enable walrus' redundant-LDWEIGHTS elision for this kernel's compile.
    Both matmul stages reuse one stationary operand across sub-segments, so
    half the weight loads are no-ops; the conservative default leaves them
    in.  Correctness is verified against the reference output downstream."""

    def __enter__(self):
        import concourse.bass_utils as bu

        self._orig = bu.run_command

        def patched(argv, **kw):
            argv = [
                a.replace("--enable-ldw-opt=false", "--enable-ldw-opt=true")
                if isinstance(a, str)
                else a
                for a in argv
            ]
            return self._orig(argv, **kw)

        bu.run_command = patched
        return self

    def __exit__(self, *exc):
        import concourse.bass_utils as bu

        bu.run_command = self._orig


def run(
    inputs: dict,
    dtype: str = "f32r",
    trace: bool = False,
    ldw_opt: bool = False,
    **run_kwargs,
):
    in_maps, L, s2, half, B = _prepare(
        inputs["x"],
        inputs["batch_indices"],
        inputs["W0_0"],
        inputs["W1_0"],
        inputs["W_out"],
        inputs["batch_size"],
        dtype=dtype,
    )
    nc = _build_program(L, s2, dtype)
    import contextlib

    with _LdwOpt() if ldw_opt else contextlib.nullcontext():
        res = run_bass_kernel_spmd(
            nc, in_maps, core_ids=list(range(N_CORES)), trace=trace, **run_kwargs
        )
    out = _assemble(res.results, half, B)
    return out, res


def kernel(**inputs) -> np.ndarray:
    out, _ = run(inputs)
    return out
